# revision 13
# baseline (speedup 1.0000x reference)
"""NoisyNet dense layer (training mode) on 8 TRN2 NeuronCores.

out[b,u] = x @ W_mu + eps_out * ((x*eps_in) @ W_sigma) + bias_mu + bias_sigma*eps_out

Sharding: data-parallel over batch (4096 -> 512 rows/core), weights/biases
replicated. On-device math runs in a transposed layout ([D,B]/[U,B]) so the
contraction dim D lands on SBUF partitions; the host does the (free)
transposes, dtype casts and the final gather.

v4 vs baseline:
 - Noise GEMM runs F8=5 of its 8 256-deep contraction chunks as fp8e4
   DoubleRow matmuls (2 k-tiles per PE pass, full double throughput
   measured); the rest stays bf16. Seed-0 rel err 1.851e-2 < 2e-2 gate
   (full fp8 would be 2.3e-2). Scales: host sends eps_in/4 and
   W_sigma*2048; PSUM holds 512*noise; epilogue ACT applies
   2^-9*psum + bias_sigma in one op. Phase-2 u-tiles take 11 PE slots
   instead of 16.
 - Every DMA chunk gets its OWN SBUF tile: multiple dma_starts into one
   tile serialize (write-write ordering) and coarse batched semaphores
   then make unrelated consumers wait on the LAST write - this was worth
   ~20us of false stalls.
 - All DMA uses >=2KB per-partition-contiguous runs (4KB packets where
   possible); small-packet transfers halve queue throughput.
 - Traffic is spread over the 3 DMA-capable queues in deadline order.
 - Output is written bf16 (host casts to fp32) in u-pair DMAs on the
   otherwise-idle sync queue, overlapping phase 2.
"""

import numpy as np
import ml_dtypes

import concourse.bacc as bacc
import concourse.mybir as mybir
import concourse.tile as tile
from concourse.bass_utils import run_bass_kernel_spmd

N_CORES = 8
B, D, U = 4096, 2048, 2048
BL = B // N_CORES          # 512 batch rows per core
P = 128                    # partitions
KT = D // P                # 16 contraction tiles of 128
UT = U // P                # 16 output tiles of 128
F8 = 5                     # DoubleRow fp8 chunks (256 contraction each)
KB0 = 2 * F8               # first bf16 k-tile of the noise GEMM
KBN = KT - KB0             # bf16 k-tiles in the noise GEMM
BF16 = mybir.dt.bfloat16
FP32 = mybir.dt.float32
FP8 = mybir.dt.float8e4
DR = mybir.MatmulPerfMode.DoubleRow
IDENT = mybir.ActivationFunctionType.Identity

_NBF = ml_dtypes.bfloat16
_NF8 = ml_dtypes.float8_e4m3   # IEEE-style e4m3, max +-240 == TRN FP8_EXP4

_cached = None


def _build():
    nc = bacc.Bacc("TRN2", target_bir_lowering=False, debug=False)

    # activations laid out [P, KT, BL]: partition p holds d = k*128+p
    xT = nc.declare_dram_parameter("xT", [P, KT, BL], BF16, isOutput=False)
    ei4T = nc.declare_dram_parameter("ei4T", [P, KT, BL], BF16, isOutput=False)
    eoT = nc.declare_dram_parameter("eoT", [P, UT, BL], BF16, isOutput=False)
    wmu = nc.declare_dram_parameter("wmu", [UT, P, KT * P], BF16, isOutput=False)
    # W_sigma*2048, partition-contiguous across u for big DMA runs:
    # fp8 DoubleRow part [p, u, kt, i, m] (d = kt*256 + i*128 + p) ...
    ws8 = nc.declare_dram_parameter("ws8", [P, UT, F8, 2, P], FP8, isOutput=False)
    # ... and bf16 tail part for k-tiles KB0..15
    wsb = nc.declare_dram_parameter("wsb", [P, UT, KBN * P], BF16, isOutput=False)
    bmu = nc.declare_dram_parameter("bmu", [P, UT], FP32, isOutput=False)
    bsg = nc.declare_dram_parameter("bsg", [P, UT], FP32, isOutput=False)
    outT = nc.declare_dram_parameter("outT", [P, UT, BL], BF16, isOutput=True)

    with tile.TileContext(nc) as tc:
        with (
            tc.tile_pool(name="acts", bufs=1) as acts,
            tc.tile_pool(name="wm", bufs=7) as wmp,
            tc.tile_pool(name="bias", bufs=1) as bp,
            tc.tile_pool(name="psum", bufs=4, space="PSUM") as pp,
            tc.tile_pool(name="psumn", bufs=4, space="PSUM") as ppn,
            tc.tile_pool(name="mean", bufs=1) as mp,
            tc.tile_pool(name="tmp", bufs=2) as tp,
            tc.tile_pool(name="out", bufs=3) as op,
        ):
            # HAM warm-up: matmuls on zeroed SBUF during the initial DMA wait
            # so the first real matmuls run closer to 2.4 GHz.
            warm_in = bp.tile([P, BL], BF16, tag="warmin")
            nc.gpsimd.memset(warm_in[:], 0.0)
            warm_ps = ppn.tile([P, BL], FP32, tag="psn")
            for _ in range(8):
                nc.tensor.matmul(warm_ps[:], warm_in[:, :P], warm_in[:])

            # ---- input streams: one tile per DMA chunk -------------------
            x_c = [acts.tile([P, 4, BL], BF16, tag=f"x{c}", name=f"x{c}")
                   for c in range(4)]
            ei_c = [acts.tile([P, 4, BL], BF16, tag=f"ei{c}", name=f"ei{c}")
                    for c in range(4)]
            eo_c = [acts.tile([P, 4, BL], BF16, tag=f"eo{c}", name=f"eo{c}")
                    for c in range(4)]
            z8_sb = acts.tile([P, F8, 2, BL], FP8, tag="z8")
            w8_h = [acts.tile([P, UT // 2, F8, 2, P], FP8, tag=f"w8{h}",
                               name=f"w8{h}") for h in range(2)]
            wb_h = [acts.tile([P, UT // 2, KBN * P], BF16, tag=f"wb{h}",
                               name=f"wb{h}") for h in range(2)]

            def xk(k):   # x k-tile view
                return x_c[k // 4][:, k % 4, :]

            def zbk(j):  # bf16 z-tail k-tile view (in ei_c, in-place product)
                return ei_c[j // 4][:, j % 4, :]

            wm_tiles = {}

            def xch(c, q):
                q.dma_start(x_c[c][:], xT[:, 4 * c:4 * c + 4, :])

            def eich(c, q):
                q.dma_start(ei_c[c][:], ei4T[:, 4 * c:4 * c + 4, :])

            def eoch(c, q):
                q.dma_start(eo_c[c][:], eoT[:, 4 * c:4 * c + 4, :])

            def fetch_wm(u, q):
                wm = wmp.tile([P, KT * P], BF16, tag="wm")
                q.dma_start(wm[:], wmu[u])
                wm_tiles[u] = wm

            # Queue plans (issue order == transfer order, all >=2KB runs):
            # sync:   wm0a c2 wm0b wm2 | wm4 wm6 wm8 wm10 wm12 eo0 wm14
            #         w8[8:] wb[8:] | out pairs (phase 2)
            # gpsimd: c1 bias wm1 wm3 | wm5 wm7 wm9 wm11 wm13 wm15
            #         w8[:8] wb[:8]
            # scalar: c0 c3 ei*4 eo1 eo2 eo3 | ACTs
            wm0 = wmp.tile([P, KT * P], BF16, tag="wm")
            nc.sync.dma_start(wm0[:, :4 * P], wmu[0][:, :4 * P])
            wm_tiles[0] = wm0
            xch(0, nc.scalar)
            xch(1, nc.gpsimd)
            xch(2, nc.sync)
            xch(3, nc.scalar)
            nc.sync.dma_start(wm0[:, 4 * P:], wmu[0][:, 4 * P:])
            bmu_t = bp.tile([P, UT], FP32, tag="bmu")
            nc.gpsimd.dma_start(bmu_t[:], bmu[:])
            bsg_t = bp.tile([P, UT], FP32, tag="bsg")
            nc.gpsimd.dma_start(bsg_t[:], bsg[:])
            fetch_wm(1, nc.gpsimd)
            fetch_wm(2, nc.sync)
            fetch_wm(3, nc.gpsimd)

            for c in range(4):
                eich(c, nc.scalar)
            eoch(1, nc.scalar)
            eoch(2, nc.scalar)
            eoch(3, nc.scalar)

            # z tiles on DVE as soon as x+eps_in chunks land:
            # z8[:,kt,:,:] (fp8) covers k-tiles 2kt..2kt+1; bf16 z tail is
            # computed in place into its ei chunk tile.
            for kt in range(F8):
                k = 2 * kt
                assert k // 4 == (k + 1) // 4
                c, r = k // 4, k % 4
                nc.vector.tensor_mul(z8_sb[:, kt, :, :],
                                     x_c[c][:, r:r + 2, :],
                                     ei_c[c][:, r:r + 2, :])
            j = KB0
            while j < KT:
                c, r = j // 4, j % 4
                r2 = 4
                nc.vector.tensor_mul(ei_c[c][:, r:r2, :],
                                     x_c[c][:, r:r2, :], ei_c[c][:, r:r2, :])
                j += r2 - r

            # remaining fetches: emission points inside the phase-1 loop
            # bound the per-engine issue order; transfers follow queue order.
            def emit(it):
                for kind, a, qn in sched.get(it, []):
                    q = getattr(nc, qn)
                    if kind == "wm":
                        fetch_wm(a, q)
                    elif kind == "w8":
                        q.dma_start(w8_h[a][:], ws8[:, 8 * a:8 * a + 8, :, :, :])
                    elif kind == "wb":
                        q.dma_start(wb_h[a][:], wsb[:, 8 * a:8 * a + 8, :])
                    else:
                        eoch(a, q)

            sched = {
                0: [("wm", 4, "sync")],
                1: [("wm", 5, "gpsimd")],
                2: [("wm", 6, "sync")],
                3: [("wm", 7, "gpsimd")],
                4: [("wm", 8, "sync")],
                5: [("wm", 9, "gpsimd")],
                6: [("wm", 10, "sync")],
                7: [("wm", 11, "gpsimd")],
                8: [("wm", 12, "sync"), ("eo", 0, "sync")],
                9: [("wm", 13, "gpsimd")],
                10: [("wm", 14, "sync"), ("w8", 0, "gpsimd")],
                11: [("wm", 15, "gpsimd"), ("w8", 1, "sync")],
                12: [("wb", 0, "gpsimd")],
                13: [("wb", 1, "sync")],
            }

            # ---- Phase 1: mean terms. t_m[u] = W_mu[u].T @ x + bias_mu[u] ----
            t_m = []
            for u in range(UT):
                emit(u)
                wm = wm_tiles.pop(u)
                pm = pp.tile([P, BL], FP32, tag="psm")
                for k in range(KT):
                    nc.tensor.matmul(
                        pm[:], wm[:, k * P:(k + 1) * P], xk(k),
                        start=(k == 0), stop=(k == KT - 1),
                    )
                tm = mp.tile([P, BL], BF16, tag=f"tm{u}")
                nc.scalar.activation(tm[:], pm[:], IDENT,
                                     bias=bmu_t[:, u:u + 1], scale=1.0)
                t_m.append(tm)

            # ---- Phase 2: noise terms + combine; outputs in u-pairs ----
            o_pair = None
            for u in range(UT):
                if u % 2 == 0:
                    o_pair = op.tile([P, 2, BL], BF16, tag="o")
                w8v = w8_h[u // 8][:, u % 8, :, :, :]
                wbv = wb_h[u // 8][:, u % 8, :]
                last = (u == UT - 1)
                halves = (0, BL // 2, BL) if last else (0, BL)
                for h in range(len(halves) - 1):
                    lo, hi = halves[h], halves[h + 1]
                    pn = ppn.tile([P, hi - lo], FP32, tag="psn")
                    for kt in range(F8):
                        nc.tensor.matmul(
                            pn[:], w8v[:, kt, :, :], z8_sb[:, kt, :, lo:hi],
                            start=(kt == 0), stop=False, perf_mode=DR,
                        )
                    for j in range(KBN):
                        nc.tensor.matmul(
                            pn[:], wbv[:, j * P:(j + 1) * P],
                            zbk(KB0 + j)[:, lo:hi],
                            start=(F8 == 0 and j == 0), stop=(j == KBN - 1),
                        )
                    t_n = tp.tile([P, hi - lo], BF16, tag="tn")
                    nc.scalar.activation(t_n[:], pn[:], IDENT,
                                         bias=bsg_t[:, u:u + 1], scale=2.0 ** -9)
                    pr = tp.tile([P, hi - lo], BF16, tag="pr")
                    nc.vector.tensor_mul(pr[:], t_n[:],
                                         eo_c[u // 4][:, u % 4, lo:hi])
                    nc.vector.tensor_add(o_pair[:, u % 2, lo:hi], pr[:],
                                         t_m[u][:, lo:hi])
                if u % 2 == 1:
                    nc.sync.dma_start(outT[:, u - 1:u + 1, :], o_pair[:])

    nc.compile()
    return nc


def _get_nc():
    global _cached
    if _cached is None:
        _cached = _build()
    return _cached


def host_prep(x, weight_mu, weight_sigma, bias_mu, bias_sigma, eps_in, eps_out):
    """Layout prep only: transposes, dtype casts/quantization, sharding."""
    def to_pkb(a):  # [B, D] -> per-core [P, KT, BL] (partition p holds k*128+p)
        a = np.ascontiguousarray(a.astype(_NBF))
        return [
            np.ascontiguousarray(
                a[c * BL:(c + 1) * BL].T.reshape(KT, P, BL).transpose(1, 0, 2))
            for c in range(N_CORES)
        ]

    xs = to_pkb(x)
    eis = to_pkb(eps_in * 0.25)
    eos = to_pkb(eps_out)  # same transform, u in place of k

    wb_ = weight_mu.astype(_NBF).reshape(KT, P, UT, P).transpose(2, 1, 0, 3)
    wmu_h = np.ascontiguousarray(wb_.reshape(UT, P, KT * P))

    ws = weight_sigma * 2048.0
    # fp8 DoubleRow part [p, u, kt, i, m]: d = kt*256 + i*128 + p, k-tile < KB0
    w8 = ws[:KB0 * P].reshape(F8, 2, P, UT, P).transpose(2, 3, 0, 1, 4)
    w8 = np.clip(w8, -240.0, 240.0).astype(_NF8)
    ws8_h = np.ascontiguousarray(w8)
    # bf16 tail [p, u, j*P+m] for k-tiles KB0..15
    wsb_h = np.ascontiguousarray(
        ws[KB0 * P:].astype(_NBF).reshape(KBN, P, UT, P)
        .transpose(1, 2, 0, 3).reshape(P, UT, KBN * P))
    bmu_h = np.ascontiguousarray(bias_mu.astype(np.float32).reshape(UT, P).T)
    bsg_h = np.ascontiguousarray(bias_sigma.astype(np.float32).reshape(UT, P).T)

    return [
        {
            "xT": xs[c],
            "ei4T": eis[c],
            "eoT": eos[c],
            "wmu": wmu_h,
            "ws8": ws8_h,
            "wsb": wsb_h,
            "bmu": bmu_h,
            "bsg": bsg_h,
        }
        for c in range(N_CORES)
    ]


def unshard(oc):
    # [P, UT, BL] bf16 -> [BL, U] fp32 (u = ut*128 + p)
    return np.asarray(oc).transpose(2, 1, 0).reshape(BL, U).astype(np.float32)


def kernel(x, weight_mu, weight_sigma, bias_mu, bias_sigma, eps_in, eps_out,
           _trace=False):
    nc = _get_nc()
    in_maps = host_prep(x, weight_mu, weight_sigma, bias_mu, bias_sigma,
                        eps_in, eps_out)

    res = run_bass_kernel_spmd(nc, in_maps, core_ids=list(range(N_CORES)),
                               trace=_trace)
    kernel.last_result = res

    out = np.empty((B, U), dtype=np.float32)
    for c in range(N_CORES):
        out[c * BL:(c + 1) * BL] = unshard(res.results[c]["outT"])
    return out


# revision 18
# speedup vs baseline: 1.0216x; 1.0216x over previous
"""NoisyNet dense layer (training mode) on 8 TRN2 NeuronCores.

out[b,u] = x @ W_mu + eps_out * ((x*eps_in) @ W_sigma) + bias_mu + bias_sigma*eps_out

Sharding: data-parallel over batch (4096 -> 512 rows/core), weights/biases
replicated. On-device math runs in a transposed layout ([D,B]/[U,B]) so the
contraction dim D lands on SBUF partitions; the host does the (free)
transposes, dtype casts and the final gather.

v4 vs baseline:
 - Noise GEMM runs F8=5 of its 8 256-deep contraction chunks as fp8e4
   DoubleRow matmuls (2 k-tiles per PE pass, full double throughput
   measured); the rest stays bf16. Seed-0 rel err 1.851e-2 < 2e-2 gate
   (full fp8 would be 2.3e-2). Scales: host sends eps_in/4 and
   W_sigma*2048; PSUM holds 512*noise; epilogue ACT applies
   2^-9*psum + bias_sigma in one op. Phase-2 u-tiles take 11 PE slots
   instead of 16.
 - Every DMA chunk gets its OWN SBUF tile: multiple dma_starts into one
   tile serialize (write-write ordering) and coarse batched semaphores
   then make unrelated consumers wait on the LAST write - this was worth
   ~20us of false stalls.
 - All DMA uses >=2KB per-partition-contiguous runs (4KB packets where
   possible); small-packet transfers halve queue throughput.
 - Traffic is spread over the 3 DMA-capable queues in deadline order.
 - Output is written bf16 (host casts to fp32) in u-pair DMAs on the
   otherwise-idle sync queue, overlapping phase 2.
"""

import numpy as np
import ml_dtypes

import concourse.bacc as bacc
import concourse.mybir as mybir
import concourse.tile as tile
from concourse.bass_utils import run_bass_kernel_spmd

N_CORES = 8
B, D, U = 4096, 2048, 2048
BL = B // N_CORES          # 512 batch rows per core
P = 128                    # partitions
KT = D // P                # 16 contraction tiles of 128
UT = U // P                # 16 output tiles of 128
F8 = 5                     # DoubleRow fp8 chunks (256 contraction each)
KB0 = 2 * F8               # first bf16 k-tile of the noise GEMM
KBN = KT - KB0             # bf16 k-tiles in the noise GEMM
BF16 = mybir.dt.bfloat16
FP32 = mybir.dt.float32
FP8 = mybir.dt.float8e4
DR = mybir.MatmulPerfMode.DoubleRow
IDENT = mybir.ActivationFunctionType.Identity

_NBF = ml_dtypes.bfloat16
_NF8 = ml_dtypes.float8_e4m3   # IEEE-style e4m3, max +-240 == TRN FP8_EXP4

_cached = None


def _build():
    nc = bacc.Bacc("TRN2", target_bir_lowering=False, debug=False)

    # activations laid out [P, KT, BL]: partition p holds d = k*128+p
    xT = nc.declare_dram_parameter("xT", [P, KT, BL], BF16, isOutput=False)
    ei4T = nc.declare_dram_parameter("ei4T", [P, KT, BL], BF16, isOutput=False)
    eoT = nc.declare_dram_parameter("eoT", [P, UT, BL], BF16, isOutput=False)
    wmu = nc.declare_dram_parameter("wmu", [UT, P, KT * P], BF16, isOutput=False)
    # W_sigma*2048, partition-contiguous across u for big DMA runs:
    # fp8 DoubleRow part [p, u, kt, i, m] (d = kt*256 + i*128 + p) ...
    ws8 = nc.declare_dram_parameter("ws8", [P, UT, F8, 2, P], FP8, isOutput=False)
    # ... and bf16 tail part for k-tiles KB0..15
    wsb = nc.declare_dram_parameter("wsb", [P, UT, KBN * P], BF16, isOutput=False)
    bmu = nc.declare_dram_parameter("bmu", [P, UT], FP32, isOutput=False)
    bsg = nc.declare_dram_parameter("bsg", [P, UT], FP32, isOutput=False)
    outT = nc.declare_dram_parameter("outT", [P, UT, BL], BF16, isOutput=True)

    with tile.TileContext(nc) as tc:
        with (
            tc.tile_pool(name="acts", bufs=1) as acts,
            tc.tile_pool(name="wm", bufs=7) as wmp,
            tc.tile_pool(name="bias", bufs=1) as bp,
            tc.tile_pool(name="psum", bufs=4, space="PSUM") as pp,
            tc.tile_pool(name="psumn", bufs=4, space="PSUM") as ppn,
            tc.tile_pool(name="mean", bufs=1) as mp,
            tc.tile_pool(name="tmp", bufs=2) as tp,
            tc.tile_pool(name="out", bufs=3) as op,
        ):
            # HAM warm-up: matmuls on zeroed SBUF during the initial DMA wait
            # so the first real matmuls run closer to 2.4 GHz.
            warm_in = bp.tile([P, BL], BF16, tag="warmin")
            nc.gpsimd.memset(warm_in[:], 0.0)
            warm_ps = ppn.tile([P, BL], FP32, tag="psn")
            for _ in range(8):
                nc.tensor.matmul(warm_ps[:], warm_in[:, :P], warm_in[:])

            # ---- input streams: one tile per DMA chunk -------------------
            x_c = [acts.tile([P, 4, BL], BF16, tag=f"x{c}", name=f"x{c}")
                   for c in range(4)]
            ei_c = [acts.tile([P, 4, BL], BF16, tag=f"ei{c}", name=f"ei{c}")
                    for c in range(4)]
            eo_c = [acts.tile([P, 4, BL], BF16, tag=f"eo{c}", name=f"eo{c}")
                    for c in range(4)]
            z8_sb = acts.tile([P, F8, 2, BL], FP8, tag="z8")
            w8_h = [acts.tile([P, UT // 2, F8, 2, P], FP8, tag=f"w8{h}",
                               name=f"w8{h}") for h in range(2)]
            wb_h = [acts.tile([P, UT // 2, KBN * P], BF16, tag=f"wb{h}",
                               name=f"wb{h}") for h in range(2)]

            def xk(k):   # x k-tile view
                return x_c[k // 4][:, k % 4, :]

            def zbk(j):  # bf16 z-tail k-tile view (in ei_c, in-place product)
                return ei_c[j // 4][:, j % 4, :]

            wm_tiles = {}

            def xch(c, q):
                q.dma_start(x_c[c][:], xT[:, 4 * c:4 * c + 4, :])

            def eich(c, q):
                q.dma_start(ei_c[c][:], ei4T[:, 4 * c:4 * c + 4, :])

            def eoch(c, q):
                q.dma_start(eo_c[c][:], eoT[:, 4 * c:4 * c + 4, :])

            def fetch_wm(u, q):
                wm = wmp.tile([P, KT * P], BF16, tag="wm")
                q.dma_start(wm[:], wmu[u])
                wm_tiles[u] = wm

            # Queue plans (issue order == transfer order, all >=2KB runs).
            # The scalar ENGINE gets only 6 quick pre-loop issues: DMA-issue
            # semaphores rotate ~3-4 deep, so long issue chains stall the
            # engine and delay the phase-1 ACTs that drain PSUM.
            # sync:   wm0a c2 wm0b wm2 | wm evens | w8[8:] | out pairs
            # gpsimd: c1 bias wm1 wm3 ei2 | wm odds, ei3, eo2, eo3, wb[8:]
            # scalar: c0 c3 ei0 ei1 eo0 eo1 | w8[:8]@u3 wb[:8]@u5 | ACTs
            wm0 = wmp.tile([P, KT * P], BF16, tag="wm")
            nc.sync.dma_start(wm0[:, :4 * P], wmu[0][:, :4 * P])
            wm_tiles[0] = wm0
            xch(0, nc.scalar)
            xch(1, nc.gpsimd)
            xch(2, nc.sync)
            xch(3, nc.scalar)
            nc.sync.dma_start(wm0[:, 4 * P:], wmu[0][:, 4 * P:])
            bmu_t = bp.tile([P, UT], FP32, tag="bmu")
            nc.gpsimd.dma_start(bmu_t[:], bmu[:])
            bsg_t = bp.tile([P, UT], FP32, tag="bsg")
            nc.gpsimd.dma_start(bsg_t[:], bsg[:])
            fetch_wm(1, nc.gpsimd)
            fetch_wm(2, nc.sync)
            fetch_wm(3, nc.gpsimd)
            eich(0, nc.scalar)
            eich(1, nc.scalar)
            eich(2, nc.gpsimd)
            eoch(0, nc.scalar)
            eoch(1, nc.scalar)

            # z tiles on DVE as soon as x+eps_in chunks land:
            # z8[:,kt,:,:] (fp8) covers k-tiles 2kt..2kt+1; bf16 z tail is
            # computed in place into its ei chunk tile. Ops for a chunk must
            # be emitted AFTER that chunk's DMA issue (program order builds
            # the dependency graph) - chunk 3's op is emitted in the loop.
            def z_ops(c):
                lo, hi = 4 * c, 4 * c + 4
                for kt in range(F8):
                    k = 2 * kt
                    if lo <= k < hi:
                        r = k % 4
                        nc.vector.tensor_mul(z8_sb[:, kt, :, :],
                                             x_c[c][:, r:r + 2, :],
                                             ei_c[c][:, r:r + 2, :])
                j = max(KB0, lo)
                if j < hi:
                    r = j % 4
                    nc.vector.tensor_mul(ei_c[c][:, r:4, :],
                                         x_c[c][:, r:4, :], ei_c[c][:, r:4, :])

            z_ops(0)
            z_ops(1)
            z_ops(2)

            # remaining fetches: emission points inside the phase-1 loop
            # bound the per-engine issue order; transfers follow queue order.
            def emit(it):
                for kind, a, qn in sched.get(it, []):
                    q = getattr(nc, qn)
                    if kind == "wm":
                        fetch_wm(a, q)
                    elif kind == "w8":
                        q.dma_start(w8_h[a][:], ws8[:, 8 * a:8 * a + 8, :, :, :])
                    elif kind == "wb":
                        q.dma_start(wb_h[a][:], wsb[:, 8 * a:8 * a + 8, :])
                    elif kind == "ei":
                        eich(a, q)
                        z_ops(a)
                    else:
                        eoch(a, q)

            sched = {
                0: [("wm", 4, "sync")],
                1: [("wm", 5, "gpsimd")],
                2: [("wm", 6, "sync")],
                3: [("wm", 7, "gpsimd"), ("w8", 0, "scalar")],
                4: [("wm", 8, "sync"), ("ei", 3, "gpsimd")],
                5: [("wm", 9, "gpsimd"), ("wb", 0, "scalar")],
                6: [("wm", 10, "sync")],
                7: [("wm", 11, "gpsimd")],
                8: [("wm", 12, "sync")],
                9: [("wm", 13, "gpsimd")],
                10: [("wm", 14, "sync")],
                11: [("wm", 15, "gpsimd"), ("w8", 1, "sync")],
                12: [("eo", 2, "gpsimd")],
                13: [("eo", 3, "gpsimd")],
                14: [("wb", 1, "gpsimd")],
            }

            # ---- Phase 1: mean terms. t_m[u] = W_mu[u].T @ x + bias_mu[u] ----
            t_m = []
            for u in range(UT):
                emit(u)
                wm = wm_tiles.pop(u)
                pm = pp.tile([P, BL], FP32, tag="psm")
                for k in range(KT):
                    nc.tensor.matmul(
                        pm[:], wm[:, k * P:(k + 1) * P], xk(k),
                        start=(k == 0), stop=(k == KT - 1),
                    )
                tm = mp.tile([P, BL], BF16, tag=f"tm{u}")
                nc.scalar.activation(tm[:], pm[:], IDENT,
                                     bias=bmu_t[:, u:u + 1], scale=1.0)
                t_m.append(tm)

            # ---- Phase 2: noise terms + combine; outputs in u-pairs ----
            o_pair = None
            for u in range(UT):
                if u % 2 == 0:
                    o_pair = op.tile([P, 2, BL], BF16, tag="o")
                w8v = w8_h[u // 8][:, u % 8, :, :, :]
                wbv = wb_h[u // 8][:, u % 8, :]
                last = (u == UT - 1)
                halves = (0, BL // 2, BL) if last else (0, BL)
                for h in range(len(halves) - 1):
                    lo, hi = halves[h], halves[h + 1]
                    pn = ppn.tile([P, hi - lo], FP32, tag="psn")
                    for kt in range(F8):
                        nc.tensor.matmul(
                            pn[:], w8v[:, kt, :, :], z8_sb[:, kt, :, lo:hi],
                            start=(kt == 0), stop=False, perf_mode=DR,
                        )
                    for j in range(KBN):
                        nc.tensor.matmul(
                            pn[:], wbv[:, j * P:(j + 1) * P],
                            zbk(KB0 + j)[:, lo:hi],
                            start=(F8 == 0 and j == 0), stop=(j == KBN - 1),
                        )
                    t_n = tp.tile([P, hi - lo], BF16, tag="tn")
                    nc.scalar.activation(t_n[:], pn[:], IDENT,
                                         bias=bsg_t[:, u:u + 1], scale=2.0 ** -9)
                    pr = tp.tile([P, hi - lo], BF16, tag="pr")
                    nc.vector.tensor_mul(pr[:], t_n[:],
                                         eo_c[u // 4][:, u % 4, lo:hi])
                    nc.vector.tensor_add(o_pair[:, u % 2, lo:hi], pr[:],
                                         t_m[u][:, lo:hi])
                if u % 2 == 1:
                    nc.sync.dma_start(outT[:, u - 1:u + 1, :], o_pair[:])

    nc.compile()
    return nc


def _get_nc():
    global _cached
    if _cached is None:
        _cached = _build()
    return _cached


def host_prep(x, weight_mu, weight_sigma, bias_mu, bias_sigma, eps_in, eps_out):
    """Layout prep only: transposes, dtype casts/quantization, sharding."""
    def to_pkb(a):  # [B, D] -> per-core [P, KT, BL] (partition p holds k*128+p)
        a = np.ascontiguousarray(a.astype(_NBF))
        return [
            np.ascontiguousarray(
                a[c * BL:(c + 1) * BL].T.reshape(KT, P, BL).transpose(1, 0, 2))
            for c in range(N_CORES)
        ]

    xs = to_pkb(x)
    eis = to_pkb(eps_in * 0.25)
    eos = to_pkb(eps_out)  # same transform, u in place of k

    wb_ = weight_mu.astype(_NBF).reshape(KT, P, UT, P).transpose(2, 1, 0, 3)
    wmu_h = np.ascontiguousarray(wb_.reshape(UT, P, KT * P))

    ws = weight_sigma * 2048.0
    # fp8 DoubleRow part [p, u, kt, i, m]: d = kt*256 + i*128 + p, k-tile < KB0
    w8 = ws[:KB0 * P].reshape(F8, 2, P, UT, P).transpose(2, 3, 0, 1, 4)
    w8 = np.clip(w8, -240.0, 240.0).astype(_NF8)
    ws8_h = np.ascontiguousarray(w8)
    # bf16 tail [p, u, j*P+m] for k-tiles KB0..15
    wsb_h = np.ascontiguousarray(
        ws[KB0 * P:].astype(_NBF).reshape(KBN, P, UT, P)
        .transpose(1, 2, 0, 3).reshape(P, UT, KBN * P))
    bmu_h = np.ascontiguousarray(bias_mu.astype(np.float32).reshape(UT, P).T)
    bsg_h = np.ascontiguousarray(bias_sigma.astype(np.float32).reshape(UT, P).T)

    return [
        {
            "xT": xs[c],
            "ei4T": eis[c],
            "eoT": eos[c],
            "wmu": wmu_h,
            "ws8": ws8_h,
            "wsb": wsb_h,
            "bmu": bmu_h,
            "bsg": bsg_h,
        }
        for c in range(N_CORES)
    ]


def unshard(oc):
    # [P, UT, BL] bf16 -> [BL, U] fp32 (u = ut*128 + p)
    return np.asarray(oc).transpose(2, 1, 0).reshape(BL, U).astype(np.float32)


def kernel(x, weight_mu, weight_sigma, bias_mu, bias_sigma, eps_in, eps_out,
           _trace=False):
    nc = _get_nc()
    in_maps = host_prep(x, weight_mu, weight_sigma, bias_mu, bias_sigma,
                        eps_in, eps_out)

    res = run_bass_kernel_spmd(nc, in_maps, core_ids=list(range(N_CORES)),
                               trace=_trace)
    kernel.last_result = res

    out = np.empty((B, U), dtype=np.float32)
    for c in range(N_CORES):
        out[c * BL:(c + 1) * BL] = unshard(res.results[c]["outT"])
    return out
